# revision 47
# baseline (speedup 1.0000x reference)
"""Multi-head self-attention (B=4, T=2048, C=1024, H=16) on 8 Trainium2 cores.

Sharding (head-split): core c handles batch b = c//2 and head-half
hh = c%2 (8 of the 16 heads), ALL 2048 queries and keys of its batch.
No K/V projection redundancy. The output projection contracts only this
core's 512 feature columns, so each core returns a PARTIAL [2048, 1024]
fp32 product; the host sums the two partials per batch and adds bo.

Engine plan (measured: bf16 N=512 matmul back-to-back at 216 ns with
LDWEIGHTS hidden; K=64 matmul pairs at tile_position (0,0)/(64,0) run
CONCURRENTLY; ScalarE ACTIVATE = (N+352)/1.2 ns, dtype-independent):
  - ScalarE exp() of the 33.5M logits is the pacer: 256 x [128,1024]
    ACTIVATEs ~ 294 us.
  - PE: V projection upfront; K^T/Q^T of head pair hp+1 and the output
    projection of hp-1 are INTERLEAVED into hp's attention inner loop so
    the PE never idles long enough for the HAM activity monitor to
    re-throttle the clock, and no separate projection phases remain.
  - DVE: bias adds, PSUM->SBUF casts, softmax normalize.

Layouts are feature-on-partition throughout: X^T via DMA transpose (sync
queue ONLY - transpose on the Activation queue loses the completion
ordering and races); K^T/Q^T per head pair (2x64 features on partitions
0:63/64:127); V as [key-chunk, head, 64+ones] so softmax denominators
ride along row 64 of the AV accumulation.
"""
import sys

sys.path.insert(0, "/opt/trn_rl_repo")

from contextlib import ExitStack

import numpy as np

import concourse.bacc as bacc
import concourse.tile as tile
from concourse import library_config, mybir
from concourse.bass_utils import run_bass_kernel_spmd

F32 = mybir.dt.float32
BF16 = mybir.dt.bfloat16
AF = mybir.ActivationFunctionType

T, C, NH, D = 2048, 1024, 16, 64
HH = 8                  # heads per core
HF = HH * D             # 512 feature columns per core
P = 128
N_KC = C // P           # 8 contraction chunks
N_TT = T // P           # 16 token/key chunks
N_HP = HH // 2          # 4 head pairs per core
N_QP = 4                # query passes of 512
QW = T // N_QP          # 512 queries per pass
VW = D + 1              # per-head V width incl. ones column

_CACHE = {}


def _build(debug=False):
    nc = bacc.Bacc("TRN2", target_bir_lowering=False, debug=False)

    x = nc.declare_dram_parameter("x", [C, T], BF16, isOutput=False)  # X^T
    wq = nc.declare_dram_parameter("wq", [C, HF], BF16, isOutput=False)
    wk = nc.declare_dram_parameter("wk", [C, HF], BF16, isOutput=False)
    wv = nc.declare_dram_parameter("wv", [C, HF], BF16, isOutput=False)
    wo = nc.declare_dram_parameter("wo", [HF, C], BF16, isOutput=False)
    bq = nc.declare_dram_parameter("bq", [HF], F32, isOutput=False)
    bv_b = nc.declare_dram_parameter("bv_b", [P, HF], F32, isOutput=False)
    out = nc.declare_dram_parameter("out", [N_TT, P, C], F32, isOutput=True)

    dbg = {}
    if debug:
        for name, shape, dt_ in [
            ("dbg_xt", [P, T], BF16),
            ("dbg_qt", [P, T], BF16),
            ("dbg_kt", [P, T], BF16),
            ("dbg_vd", [P, HH * VW], BF16),
            ("dbg_s", [P, 2 * QW], F32),
            ("dbg_pt", [P, 2 * QW], BF16),
            ("dbg_o", [VW, QW], F32),
            ("dbg_rc", [1, QW], F32),
            ("dbg_bc", [64, QW], F32),
            ("dbg_ao", [P, T], BF16),
        ]:
            dbg[name] = nc.declare_dram_parameter(name, shape, dt_, isOutput=True)

    with tile.TileContext(nc) as tc, ExitStack() as ctx:
        big = ctx.enter_context(tc.tile_pool(name="big", bufs=1))
        pt_pool = ctx.enter_context(tc.tile_pool(name="pt", bufs=3))
        rc_pool = ctx.enter_context(tc.tile_pool(name="rc", bufs=2))
        bc_pool = ctx.enter_context(tc.tile_pool(name="bc", bufs=2))
        s_ps = ctx.enter_context(tc.tile_pool(name="sps", bufs=2, space="PSUM"))
        o_ps = ctx.enter_context(tc.tile_pool(name="ops", bufs=3, space="PSUM"))
        pr_ps = ctx.enter_context(tc.tile_pool(name="prps", bufs=1, space="PSUM"))

        nc.gpsimd.load_library(library_config.attn)

        # ---- inputs to SBUF -------------------------------------------------
        # DMA ordering is the startup critical path: X^T (pre-transposed on
        # host) and Wv gate the V projection. X^T lands token-slice-major so
        # V(tt=0) unblocks after ~0.5 MB instead of the full 4 MB.
        xt = big.tile([P, N_KC, T], BF16)          # X^T (c, t)
        qdma3 = [nc.sync, nc.scalar, nc.gpsimd]
        wv_t = big.tile([P, N_KC, HF], BF16)
        wk_t = big.tile([P, N_KC, HF], BF16)
        wq_t = big.tile([P, N_KC, HF], BF16)
        wdma = [nc.scalar, nc.gpsimd]
        for kc in range(N_KC):
            wdma[kc % 2].dma_start(out=wv_t[:, kc, :], in_=wv[kc * P : (kc + 1) * P, :])
        for kc in range(N_KC):
            qdma3[kc % 3].dma_start(
                out=xt[:, kc, :], in_=x[kc * P : (kc + 1) * P, :]
            )
        for kc in range(N_KC):
            wdma[kc % 2].dma_start(out=wk_t[:, kc, :], in_=wk[kc * P : (kc + 1) * P, :])
        for kc in range(N_KC):
            wdma[kc % 2].dma_start(out=wq_t[:, kc, :], in_=wq[kc * P : (kc + 1) * P, :])

        bq_t = big.tile([P, N_HP], F32)
        for hp in range(N_HP):
            nc.sync.dma_start(
                out=bq_t[:, hp : hp + 1], in_=bq[hp * P : (hp + 1) * P].unsqueeze(-1)
            )
        bv_t = big.tile([P, HF], F32)
        nc.sync.dma_start(out=bv_t[:, :], in_=bv_b[:, :])

        wo_t = big.tile([P, N_HP, C], BF16)
        for hp in range(N_HP):
            wdma[hp % 2].dma_start(out=wo_t[:, hp, :], in_=wo[hp * P : (hp + 1) * P, :])

        v_res = big.tile([P, N_TT, HH * VW], BF16)  # [v_h | 1] per head per chunk
        kt_res = big.tile([P, N_HP, T], BF16)       # K^T (f, t)
        qt = big.tile([P, N_HP, T], BF16)           # Q^T (f, q)
        attout = big.tile([P, N_HP, T], BF16)       # normalized O^T

        v_ones = v_res.rearrange("p t (h w) -> p t h w", w=VW)
        nc.vector.memset(v_ones[:, :, :, D : D + 1], 1.0)

        # ---- V = X @ Wv + bv, all heads (tokens on partitions) --------------
        bv_v = bv_t.rearrange("p (h d) -> p h d", h=HH)
        for tt in range(N_TT):
            pvf = s_ps.tile([P, 2 * QW], F32, tag="s")
            pv = pvf[:, 0:HF]
            for kc in range(N_KC):
                nc.tensor.matmul(
                    pv[:, :],
                    xt[:, kc, tt * P : (tt + 1) * P],
                    wv_t[:, kc, :],
                    start=(kc == 0),
                    stop=(kc == N_KC - 1),
                )
            pv_v = pv.rearrange("p (h d) -> p h d", h=HH)
            nc.vector.tensor_add(v_ones[:, tt, :, 0:D], pv_v[:, :, :], bv_v[:, :, :])

        # ---- projection work generators (emitted inline with attention) ----
        def k_proj_steps(hp, pool=None, tag="pr"):
            """K^T(hp): 4 th-groups x (8 accumulating MMs + a DVE cast)."""
            for th in range(N_QP):
                pk = (pool or pr_ps).tile([P, QW], F32, tag=tag)
                for kc in range(N_KC):
                    yield lambda hp=hp, th=th, kc=kc, pk=pk: nc.tensor.matmul(
                        pk[:, :],
                        wk_t[:, kc, hp * P : (hp + 1) * P],
                        xt[:, kc, th * QW : (th + 1) * QW],
                        start=(kc == 0),
                        stop=(kc == N_KC - 1),
                    )
                yield lambda hp=hp, th=th, pk=pk: nc.vector.tensor_copy(
                    kt_res[:, hp, th * QW : (th + 1) * QW], pk[:, :]
                )

        def q_proj_steps(hp, pool=None, tag="pr"):
            for th in range(N_QP):
                pq = (pool or pr_ps).tile([P, QW], F32, tag=tag)
                for kc in range(N_KC):
                    yield lambda hp=hp, th=th, kc=kc, pq=pq: nc.tensor.matmul(
                        pq[:, :],
                        wq_t[:, kc, hp * P : (hp + 1) * P],
                        xt[:, kc, th * QW : (th + 1) * QW],
                        start=(kc == 0),
                        stop=(kc == N_KC - 1),
                    )
                yield lambda hp=hp, th=th, pq=pq: nc.vector.tensor_scalar_add(
                    qt[:, hp, th * QW : (th + 1) * QW], pq[:, :], bq_t[:, hp : hp + 1]
                )

        odma = [nc.sync, nc.scalar, nc.gpsimd]

        def out_proj_steps(qms, use_s_pool=False):
            """Output projection for query chunks qms (contract all 4 hp)."""
            for qm in qms:
                for nh in range(2):
                    if use_s_pool and (qm + nh) % 2 == 0:
                        # tail only: the s pool is idle after the last ACT
                        po_f = s_ps.tile([P, 2 * QW], F32, tag="s")
                        po = po_f[:, 0:QW]
                    else:
                        po = pr_ps.tile([P, QW], F32, tag="pr")
                    for hp in range(N_HP):
                        yield lambda qm=qm, nh=nh, hp=hp, po=po: nc.tensor.matmul(
                            po[:, :],
                            attout[:, hp, qm * P : (qm + 1) * P],
                            wo_t[:, hp, nh * QW : (nh + 1) * QW],
                            start=(hp == 0),
                            stop=(hp == N_HP - 1),
                        )

                    def _drain(qm=qm, nh=nh, po=po):
                        os_ = bc_pool.tile([P, QW], F32, tag="os")
                        nc.vector.tensor_copy(os_[:, :], po[:, :])
                        odma[(2 * qm + nh) % 3].dma_start(
                            out=out[qm, :, nh * QW : (nh + 1) * QW], in_=os_[:, :]
                        )

                    yield _drain

        def chain(*gens):
            for g in gens:
                yield from g

        # upfront: K^T(0), Q^T(0) (V is already queued above); these use
        # the s pool (idle until attention starts)

        def _s_pool_qw():
            class p:
                @staticmethod
                def tile(shape, dt_, tag=None):
                    t = s_ps.tile([P, 2 * QW], dt_, tag="s")
                    return t[:, 0 : shape[1]]
            return p

        for step in chain(
            k_proj_steps(0, pool=_s_pool_qw()), q_proj_steps(0, pool=_s_pool_qw())
        ):
            step()

        # side work emitted during attention inner loops. The chip power
        # manager allows ~160-200 us of full-rate PE, then duty-cycles the
        # clock to ~0.686 - so ALL projection side work is front-loaded into
        # the first two head pairs (the grace window), leaving hp 2..3 pure
        # attention (~0.69 PE duty, which the clamp tolerates at full pace).
        # Out-proj needs ALL head pairs' attout, so it can only run during
        # hp 3 (pass qp covers chunks of pass qp-1) plus a tail.
        side = {}
        for hp in range(3):
            g = chain(k_proj_steps(hp + 1), q_proj_steps(hp + 1))
            for qp in range(N_QP):
                side[(hp, qp)] = (g, 72 / 64)
        side[(3, 0)] = (iter(()), 0.0)
        for qp in range(1, N_QP):
            side[(3, qp)] = (out_proj_steps(range(4 * (qp - 1), 4 * qp)), 36 / 16)

        # ---- attention: per head pair, per query pass -----------------------
        for hp in range(N_HP):
            hA, hB = 2 * hp, 2 * hp + 1
            for qp in range(N_QP):
                gen, side_per_iter = side[(hp, qp)]
                quota = 0.0
                q0 = qp * QW
                oA = o_ps.tile([VW, QW], F32, tag="o")
                oB = o_ps.tile([VW, QW], F32, tag="o")
                for kt in range(N_TT):
                    s = s_ps.tile([P, 2 * QW], F32, tag="s")
                    nc.tensor.matmul(
                        s[:, 0:QW],
                        kt_res[0:64, hp, kt * P : (kt + 1) * P],
                        qt[0:64, hp, q0 : q0 + QW],
                        start=True,
                        stop=True,
                        tile_position=(0, 0),
                    )
                    nc.tensor.matmul(
                        s[:, QW : 2 * QW],
                        kt_res[64:128, hp, kt * P : (kt + 1) * P],
                        qt[64:128, hp, q0 : q0 + QW],
                        start=True,
                        stop=True,
                        tile_position=(64, 0),
                    )
                    p_t = pt_pool.tile([P, 2 * QW], BF16, tag="pt")
                    nc.scalar.activation(p_t[:, :], s[:, :], AF.Exp, scale=0.125)
                    if debug and hp == 0 and qp == 0 and kt == 0:
                        dcp = bc_pool.tile([P, 2 * QW], F32, tag="dbgs")
                        nc.vector.tensor_copy(dcp[:, :], s[:, :])
                        nc.sync.dma_start(out=dbg["dbg_s"][:, :], in_=dcp[:, :])
                        nc.sync.dma_start(out=dbg["dbg_pt"][:, :], in_=p_t[:, :])
                    nc.tensor.matmul(
                        oA[:, :],
                        v_res[:, kt, hA * VW : (hA + 1) * VW],
                        p_t[:, 0:QW],
                        start=(kt == 0),
                        stop=(kt == N_TT - 1),
                    )
                    nc.tensor.matmul(
                        oB[:, :],
                        v_res[:, kt, hB * VW : (hB + 1) * VW],
                        p_t[:, QW : 2 * QW],
                        start=(kt == 0),
                        stop=(kt == N_TT - 1),
                    )
                    # emit interleaved projection work
                    quota += side_per_iter
                    while quota >= 1.0:
                        step = next(gen, None)
                        if step is None:
                            quota = 0.0
                            break
                        step()
                        quota -= 1.0

                if debug and hp == 0 and qp == 0:
                    ocp = bc_pool.tile([VW, QW], F32, tag="dbgo")
                    nc.vector.tensor_copy(ocp[:, :], oA[:, :])
                    nc.sync.dma_start(out=dbg["dbg_o"][:, :], in_=ocp[:, :])
                # normalize: attout[d, q] = O[d, q] / O[64, q].
                # Copy O and den out of PSUM FIRST so the O banks free after
                # ~1.2us (the next pass's AV accumulation reuses them); the
                # recip -> gpsimd-broadcast -> mul chain then runs off the
                # PSUM critical path entirely.
                chains = []
                for row0, o_t in ((0, oA), (64, oB)):
                    o_sb = bc_pool.tile([64, QW], F32, tag="osb")
                    nc.vector.tensor_copy(o_sb[:, :], o_t[0:64, :])
                    den_t = rc_pool.tile([1, QW], F32, tag="den")
                    nc.vector.tensor_copy(den_t[:, :], o_t[64:VW, :])
                    chains.append((row0, o_sb, den_t))
                for row0, o_sb, den_t in chains:
                    rc_t = rc_pool.tile([1, QW], F32, tag="rc")
                    nc.vector.reciprocal_approx_fast(out=rc_t[:, :], in_=den_t[:, :])
                    bc_t = bc_pool.tile([64, QW], F32, tag="bc")
                    nc.gpsimd.partition_broadcast(bc_t[:, :], rc_t[:, :])
                    nc.vector.tensor_mul(
                        attout[row0 : row0 + 64, hp, q0 : q0 + QW],
                        o_sb[:, :],
                        bc_t[:, :],
                    )
                    if debug and hp == 0 and qp == 0 and row0 == 0:
                        nc.sync.dma_start(out=dbg["dbg_rc"][:, :], in_=rc_t[:, :])
                        nc.sync.dma_start(out=dbg["dbg_bc"][:, :], in_=bc_t[:, :])
                # drain leftover side work (shared gens span all 4 passes)
                if qp == N_QP - 1 or hp == 3:
                    for step in gen:
                        step()

        if debug:
            nc.sync.dma_start(out=dbg["dbg_xt"][:, :], in_=xt[:, 0, :])
            nc.sync.dma_start(out=dbg["dbg_qt"][:, :], in_=qt[:, 0, :])
            nc.sync.dma_start(out=dbg["dbg_kt"][:, :], in_=kt_res[:, 0, :])
            nc.sync.dma_start(out=dbg["dbg_vd"][:, :], in_=v_res[:, 0, :])
            nc.sync.dma_start(out=dbg["dbg_ao"][:, :], in_=attout[:, 0, :])

        # ---- output projection tail (qm 12..15; rest ran inside hp 3) -------
        for step in out_proj_steps(range(12, N_TT), use_s_pool=True):
            step()

    nc.finalize()
    return nc


def _get_program():
    if "nc" not in _CACHE:
        _CACHE["nc"] = _build()
    return _CACHE["nc"]


def _bf16(a):
    import ml_dtypes

    return np.asarray(a, np.float32).astype(ml_dtypes.bfloat16)


def kernel(x, Wq, bq, Wk, bk, Wv, bv, Wo, bo, _trace=False, _trace_kwargs=None):
    x = np.asarray(x, np.float32)
    bq, bv, bo = (np.asarray(b, np.float32) for b in (bq, bv, bo))
    # bk unused: a key-side bias adds a per-query constant to every logit of a
    # softmax row, which cancels exactly in the softmax.

    x_b = [np.ascontiguousarray(_bf16(x[b]).T) for b in range(4)]
    wq_h = [_bf16(Wq[:, h * HF : (h + 1) * HF]) for h in range(2)]
    wk_h = [_bf16(Wk[:, h * HF : (h + 1) * HF]) for h in range(2)]
    wv_h = [_bf16(Wv[:, h * HF : (h + 1) * HF]) for h in range(2)]
    wo_h = [np.ascontiguousarray(_bf16(Wo[h * HF : (h + 1) * HF, :])) for h in range(2)]
    bq_h = [np.ascontiguousarray(bq[h * HF : (h + 1) * HF]) for h in range(2)]
    bv_h = [
        np.ascontiguousarray(np.broadcast_to(bv[h * HF : (h + 1) * HF], (P, HF)))
        for h in range(2)
    ]

    nc = _get_program()
    in_maps = []
    for c in range(8):
        b, hh = divmod(c, 2)
        in_maps.append(
            {
                "x": x_b[b],
                "wq": wq_h[hh], "wk": wk_h[hh], "wv": wv_h[hh],
                "wo": wo_h[hh], "bq": bq_h[hh], "bv_b": bv_h[hh],
            }
        )

    kw = {}
    if _trace:
        kw = dict(trace=True, **(_trace_kwargs or {}))
    res = run_bass_kernel_spmd(nc, in_maps, list(range(8)), **kw)
    _CACHE["last_result"] = res

    outp = np.empty((4, T, C), np.float32)
    for b in range(4):
        p0 = res.results[2 * b]["out"].reshape(T, C)
        p1 = res.results[2 * b + 1]["out"].reshape(T, C)
        outp[b] = p0 + p1
    outp += bo.astype(np.float32)
    return outp


# revision 48
# speedup vs baseline: 1.0017x; 1.0017x over previous
"""Multi-head self-attention (B=4, T=2048, C=1024, H=16) on 8 Trainium2 cores.

Sharding (head-split): core c handles batch b = c//2 and head-half
hh = c%2 (8 of the 16 heads), ALL 2048 queries and keys of its batch.
No K/V projection redundancy. The output projection contracts only this
core's 512 feature columns, so each core returns a PARTIAL [2048, 1024]
fp32 product; the host sums the two partials per batch and adds bo.

Engine plan (measured: bf16 N=512 matmul back-to-back at 216 ns with
LDWEIGHTS hidden; K=64 matmul pairs at tile_position (0,0)/(64,0) run
CONCURRENTLY; ScalarE ACTIVATE = (N+352)/1.2 ns, dtype-independent):
  - ScalarE exp() of the 33.5M logits is the pacer: 256 x [128,1024]
    ACTIVATEs ~ 294 us.
  - PE: V projection upfront; K^T/Q^T of head pair hp+1 and the output
    projection of hp-1 are INTERLEAVED into hp's attention inner loop so
    the PE never idles long enough for the HAM activity monitor to
    re-throttle the clock, and no separate projection phases remain.
  - DVE: bias adds, PSUM->SBUF casts, softmax normalize.

Layouts are feature-on-partition throughout: X^T via DMA transpose (sync
queue ONLY - transpose on the Activation queue loses the completion
ordering and races); K^T/Q^T per head pair (2x64 features on partitions
0:63/64:127); V as [key-chunk, head, 64+ones] so softmax denominators
ride along row 64 of the AV accumulation.
"""
import sys

sys.path.insert(0, "/opt/trn_rl_repo")

from contextlib import ExitStack

import numpy as np

import concourse.bacc as bacc
import concourse.tile as tile
from concourse import library_config, mybir
from concourse.bass_utils import run_bass_kernel_spmd

F32 = mybir.dt.float32
BF16 = mybir.dt.bfloat16
AF = mybir.ActivationFunctionType

T, C, NH, D = 2048, 1024, 16, 64
HH = 8                  # heads per core
HF = HH * D             # 512 feature columns per core
P = 128
N_KC = C // P           # 8 contraction chunks
N_TT = T // P           # 16 token/key chunks
N_HP = HH // 2          # 4 head pairs per core
N_QP = 4                # query passes of 512
QW = T // N_QP          # 512 queries per pass
VW = D + 1              # per-head V width incl. ones column

_CACHE = {}


def _build(debug=False):
    nc = bacc.Bacc("TRN2", target_bir_lowering=False, debug=False)

    x = nc.declare_dram_parameter("x", [C, T], BF16, isOutput=False)  # X^T
    wq = nc.declare_dram_parameter("wq", [C, HF], BF16, isOutput=False)
    wk = nc.declare_dram_parameter("wk", [C, HF], BF16, isOutput=False)
    wv = nc.declare_dram_parameter("wv", [C, HF], BF16, isOutput=False)
    wo = nc.declare_dram_parameter("wo", [HF, C], BF16, isOutput=False)
    bq = nc.declare_dram_parameter("bq", [HF], F32, isOutput=False)
    bv_b = nc.declare_dram_parameter("bv_b", [P, HF], F32, isOutput=False)
    out = nc.declare_dram_parameter("out", [N_TT, P, C], F32, isOutput=True)

    dbg = {}
    if debug:
        for name, shape, dt_ in [
            ("dbg_xt", [P, T], BF16),
            ("dbg_qt", [P, T], BF16),
            ("dbg_kt", [P, T], BF16),
            ("dbg_vd", [P, HH * VW], BF16),
            ("dbg_s", [P, 2 * QW], F32),
            ("dbg_pt", [P, 2 * QW], BF16),
            ("dbg_o", [VW, QW], F32),
            ("dbg_rc", [1, QW], F32),
            ("dbg_bc", [64, QW], F32),
            ("dbg_ao", [P, T], BF16),
        ]:
            dbg[name] = nc.declare_dram_parameter(name, shape, dt_, isOutput=True)

    with tile.TileContext(nc) as tc, ExitStack() as ctx:
        big = ctx.enter_context(tc.tile_pool(name="big", bufs=1))
        pt_pool = ctx.enter_context(tc.tile_pool(name="pt", bufs=3))
        rc_pool = ctx.enter_context(tc.tile_pool(name="rc", bufs=2))
        bc_pool = ctx.enter_context(tc.tile_pool(name="bc", bufs=2))
        s_ps = ctx.enter_context(tc.tile_pool(name="sps", bufs=2, space="PSUM"))
        o_ps = ctx.enter_context(tc.tile_pool(name="ops", bufs=3, space="PSUM"))
        pr_ps = ctx.enter_context(tc.tile_pool(name="prps", bufs=1, space="PSUM"))

        nc.gpsimd.load_library(library_config.attn)

        # ---- inputs to SBUF -------------------------------------------------
        # DMA ordering is the startup critical path: X^T (pre-transposed on
        # host) and Wv gate the V projection. X^T lands token-slice-major so
        # V(tt=0) unblocks after ~0.5 MB instead of the full 4 MB.
        xt = big.tile([P, N_KC, T], BF16)          # X^T (c, t)
        qdma3 = [nc.sync, nc.scalar, nc.gpsimd]
        wv_t = big.tile([P, N_KC, HF], BF16)
        wk_t = big.tile([P, N_KC, HF], BF16)
        wq_t = big.tile([P, N_KC, HF], BF16)
        wdma = [nc.scalar, nc.gpsimd]
        # interleave Wv and X^T chunk loads so the V projection's per-kc
        # (xt, wv) pairs land together instead of xt queueing behind all of wv
        for kc in range(N_KC):
            wdma[kc % 2].dma_start(out=wv_t[:, kc, :], in_=wv[kc * P : (kc + 1) * P, :])
            qdma3[kc % 3].dma_start(
                out=xt[:, kc, :], in_=x[kc * P : (kc + 1) * P, :]
            )
        for kc in range(N_KC):
            wdma[kc % 2].dma_start(out=wk_t[:, kc, :], in_=wk[kc * P : (kc + 1) * P, :])
        for kc in range(N_KC):
            wdma[kc % 2].dma_start(out=wq_t[:, kc, :], in_=wq[kc * P : (kc + 1) * P, :])

        bq_t = big.tile([P, N_HP], F32)
        for hp in range(N_HP):
            nc.sync.dma_start(
                out=bq_t[:, hp : hp + 1], in_=bq[hp * P : (hp + 1) * P].unsqueeze(-1)
            )
        bv_t = big.tile([P, HF], F32)
        nc.sync.dma_start(out=bv_t[:, :], in_=bv_b[:, :])

        wo_t = big.tile([P, N_HP, C], BF16)
        for hp in range(N_HP):
            wdma[hp % 2].dma_start(out=wo_t[:, hp, :], in_=wo[hp * P : (hp + 1) * P, :])

        v_res = big.tile([P, N_TT, HH * VW], BF16)  # [v_h | 1] per head per chunk
        kt_res = big.tile([P, N_HP, T], BF16)       # K^T (f, t)
        qt = big.tile([P, N_HP, T], BF16)           # Q^T (f, q)
        attout = big.tile([P, N_HP, T], BF16)       # normalized O^T

        v_ones = v_res.rearrange("p t (h w) -> p t h w", w=VW)
        nc.vector.memset(v_ones[:, :, :, D : D + 1], 1.0)

        # ---- V = X @ Wv + bv, all heads (tokens on partitions) --------------
        bv_v = bv_t.rearrange("p (h d) -> p h d", h=HH)
        for tt in range(N_TT):
            pvf = s_ps.tile([P, 2 * QW], F32, tag="s")
            pv = pvf[:, 0:HF]
            for kc in range(N_KC):
                nc.tensor.matmul(
                    pv[:, :],
                    xt[:, kc, tt * P : (tt + 1) * P],
                    wv_t[:, kc, :],
                    start=(kc == 0),
                    stop=(kc == N_KC - 1),
                )
            pv_v = pv.rearrange("p (h d) -> p h d", h=HH)
            nc.vector.tensor_add(v_ones[:, tt, :, 0:D], pv_v[:, :, :], bv_v[:, :, :])

        # ---- projection work generators (emitted inline with attention) ----
        def k_proj_steps(hp, pool=None, tag="pr"):
            """K^T(hp): 4 th-groups x (8 accumulating MMs + a DVE cast)."""
            for th in range(N_QP):
                pk = (pool or pr_ps).tile([P, QW], F32, tag=tag)
                for kc in range(N_KC):
                    yield lambda hp=hp, th=th, kc=kc, pk=pk: nc.tensor.matmul(
                        pk[:, :],
                        wk_t[:, kc, hp * P : (hp + 1) * P],
                        xt[:, kc, th * QW : (th + 1) * QW],
                        start=(kc == 0),
                        stop=(kc == N_KC - 1),
                    )
                yield lambda hp=hp, th=th, pk=pk: nc.vector.tensor_copy(
                    kt_res[:, hp, th * QW : (th + 1) * QW], pk[:, :]
                )

        def q_proj_steps(hp, pool=None, tag="pr"):
            for th in range(N_QP):
                pq = (pool or pr_ps).tile([P, QW], F32, tag=tag)
                for kc in range(N_KC):
                    yield lambda hp=hp, th=th, kc=kc, pq=pq: nc.tensor.matmul(
                        pq[:, :],
                        wq_t[:, kc, hp * P : (hp + 1) * P],
                        xt[:, kc, th * QW : (th + 1) * QW],
                        start=(kc == 0),
                        stop=(kc == N_KC - 1),
                    )
                yield lambda hp=hp, th=th, pq=pq: nc.vector.tensor_scalar_add(
                    qt[:, hp, th * QW : (th + 1) * QW], pq[:, :], bq_t[:, hp : hp + 1]
                )

        odma = [nc.sync, nc.scalar, nc.gpsimd]

        def out_proj_steps(qms, use_s_pool=False):
            """Output projection for query chunks qms (contract all 4 hp)."""
            for qm in qms:
                for nh in range(2):
                    if use_s_pool and (qm + nh) % 2 == 0:
                        # tail only: the s pool is idle after the last ACT
                        po_f = s_ps.tile([P, 2 * QW], F32, tag="s")
                        po = po_f[:, 0:QW]
                    else:
                        po = pr_ps.tile([P, QW], F32, tag="pr")
                    for hp in range(N_HP):
                        yield lambda qm=qm, nh=nh, hp=hp, po=po: nc.tensor.matmul(
                            po[:, :],
                            attout[:, hp, qm * P : (qm + 1) * P],
                            wo_t[:, hp, nh * QW : (nh + 1) * QW],
                            start=(hp == 0),
                            stop=(hp == N_HP - 1),
                        )

                    def _drain(qm=qm, nh=nh, po=po):
                        os_ = bc_pool.tile([P, QW], F32, tag="os")
                        nc.vector.tensor_copy(os_[:, :], po[:, :])
                        odma[(2 * qm + nh) % 3].dma_start(
                            out=out[qm, :, nh * QW : (nh + 1) * QW], in_=os_[:, :]
                        )

                    yield _drain

        def chain(*gens):
            for g in gens:
                yield from g

        # upfront: K^T(0), Q^T(0) (V is already queued above); these use
        # the s pool (idle until attention starts)

        def _s_pool_qw():
            class p:
                @staticmethod
                def tile(shape, dt_, tag=None):
                    t = s_ps.tile([P, 2 * QW], dt_, tag="s")
                    return t[:, 0 : shape[1]]
            return p

        for step in chain(
            k_proj_steps(0, pool=_s_pool_qw()), q_proj_steps(0, pool=_s_pool_qw())
        ):
            step()

        # side work emitted during attention inner loops. The chip power
        # manager allows ~160-200 us of full-rate PE, then duty-cycles the
        # clock to ~0.686 - so ALL projection side work is front-loaded into
        # the first two head pairs (the grace window), leaving hp 2..3 pure
        # attention (~0.69 PE duty, which the clamp tolerates at full pace).
        # Out-proj needs ALL head pairs' attout, so it can only run during
        # hp 3 (pass qp covers chunks of pass qp-1) plus a tail.
        side = {}
        for hp in range(3):
            g = chain(k_proj_steps(hp + 1), q_proj_steps(hp + 1))
            for qp in range(N_QP):
                side[(hp, qp)] = (g, 72 / 64)
        side[(3, 0)] = (iter(()), 0.0)
        for qp in range(1, N_QP):
            side[(3, qp)] = (out_proj_steps(range(4 * (qp - 1), 4 * qp)), 36 / 16)

        # ---- attention: per head pair, per query pass -----------------------
        for hp in range(N_HP):
            hA, hB = 2 * hp, 2 * hp + 1
            for qp in range(N_QP):
                gen, side_per_iter = side[(hp, qp)]
                quota = 0.0
                q0 = qp * QW
                oA = o_ps.tile([VW, QW], F32, tag="o")
                oB = o_ps.tile([VW, QW], F32, tag="o")
                for kt in range(N_TT):
                    s = s_ps.tile([P, 2 * QW], F32, tag="s")
                    nc.tensor.matmul(
                        s[:, 0:QW],
                        kt_res[0:64, hp, kt * P : (kt + 1) * P],
                        qt[0:64, hp, q0 : q0 + QW],
                        start=True,
                        stop=True,
                        tile_position=(0, 0),
                    )
                    nc.tensor.matmul(
                        s[:, QW : 2 * QW],
                        kt_res[64:128, hp, kt * P : (kt + 1) * P],
                        qt[64:128, hp, q0 : q0 + QW],
                        start=True,
                        stop=True,
                        tile_position=(64, 0),
                    )
                    p_t = pt_pool.tile([P, 2 * QW], BF16, tag="pt")
                    nc.scalar.activation(p_t[:, :], s[:, :], AF.Exp, scale=0.125)
                    if debug and hp == 0 and qp == 0 and kt == 0:
                        dcp = bc_pool.tile([P, 2 * QW], F32, tag="dbgs")
                        nc.vector.tensor_copy(dcp[:, :], s[:, :])
                        nc.sync.dma_start(out=dbg["dbg_s"][:, :], in_=dcp[:, :])
                        nc.sync.dma_start(out=dbg["dbg_pt"][:, :], in_=p_t[:, :])
                    nc.tensor.matmul(
                        oA[:, :],
                        v_res[:, kt, hA * VW : (hA + 1) * VW],
                        p_t[:, 0:QW],
                        start=(kt == 0),
                        stop=(kt == N_TT - 1),
                    )
                    nc.tensor.matmul(
                        oB[:, :],
                        v_res[:, kt, hB * VW : (hB + 1) * VW],
                        p_t[:, QW : 2 * QW],
                        start=(kt == 0),
                        stop=(kt == N_TT - 1),
                    )
                    # emit interleaved projection work
                    quota += side_per_iter
                    while quota >= 1.0:
                        step = next(gen, None)
                        if step is None:
                            quota = 0.0
                            break
                        step()
                        quota -= 1.0

                if debug and hp == 0 and qp == 0:
                    ocp = bc_pool.tile([VW, QW], F32, tag="dbgo")
                    nc.vector.tensor_copy(ocp[:, :], oA[:, :])
                    nc.sync.dma_start(out=dbg["dbg_o"][:, :], in_=ocp[:, :])
                # normalize: attout[d, q] = O[d, q] / O[64, q].
                # Copy O and den out of PSUM FIRST so the O banks free after
                # ~1.2us (the next pass's AV accumulation reuses them); the
                # recip -> gpsimd-broadcast -> mul chain then runs off the
                # PSUM critical path entirely.
                chains = []
                for row0, o_t in ((0, oA), (64, oB)):
                    o_sb = bc_pool.tile([64, QW], F32, tag="osb")
                    nc.vector.tensor_copy(o_sb[:, :], o_t[0:64, :])
                    den_t = rc_pool.tile([1, QW], F32, tag="den")
                    nc.vector.tensor_copy(den_t[:, :], o_t[64:VW, :])
                    chains.append((row0, o_sb, den_t))
                for row0, o_sb, den_t in chains:
                    rc_t = rc_pool.tile([1, QW], F32, tag="rc")
                    nc.vector.reciprocal_approx_fast(out=rc_t[:, :], in_=den_t[:, :])
                    bc_t = bc_pool.tile([64, QW], F32, tag="bc")
                    nc.gpsimd.partition_broadcast(bc_t[:, :], rc_t[:, :])
                    nc.vector.tensor_mul(
                        attout[row0 : row0 + 64, hp, q0 : q0 + QW],
                        o_sb[:, :],
                        bc_t[:, :],
                    )
                    if debug and hp == 0 and qp == 0 and row0 == 0:
                        nc.sync.dma_start(out=dbg["dbg_rc"][:, :], in_=rc_t[:, :])
                        nc.sync.dma_start(out=dbg["dbg_bc"][:, :], in_=bc_t[:, :])
                # drain leftover side work (shared gens span all 4 passes)
                if qp == N_QP - 1 or hp == 3:
                    for step in gen:
                        step()

        if debug:
            nc.sync.dma_start(out=dbg["dbg_xt"][:, :], in_=xt[:, 0, :])
            nc.sync.dma_start(out=dbg["dbg_qt"][:, :], in_=qt[:, 0, :])
            nc.sync.dma_start(out=dbg["dbg_kt"][:, :], in_=kt_res[:, 0, :])
            nc.sync.dma_start(out=dbg["dbg_vd"][:, :], in_=v_res[:, 0, :])
            nc.sync.dma_start(out=dbg["dbg_ao"][:, :], in_=attout[:, 0, :])

        # ---- output projection tail (qm 12..15; rest ran inside hp 3) -------
        for step in out_proj_steps(range(12, N_TT), use_s_pool=True):
            step()

    nc.finalize()
    return nc


def _get_program():
    if "nc" not in _CACHE:
        _CACHE["nc"] = _build()
    return _CACHE["nc"]


def _bf16(a):
    import ml_dtypes

    return np.asarray(a, np.float32).astype(ml_dtypes.bfloat16)


def kernel(x, Wq, bq, Wk, bk, Wv, bv, Wo, bo, _trace=False, _trace_kwargs=None):
    x = np.asarray(x, np.float32)
    bq, bv, bo = (np.asarray(b, np.float32) for b in (bq, bv, bo))
    # bk unused: a key-side bias adds a per-query constant to every logit of a
    # softmax row, which cancels exactly in the softmax.

    x_b = [np.ascontiguousarray(_bf16(x[b]).T) for b in range(4)]
    wq_h = [_bf16(Wq[:, h * HF : (h + 1) * HF]) for h in range(2)]
    wk_h = [_bf16(Wk[:, h * HF : (h + 1) * HF]) for h in range(2)]
    wv_h = [_bf16(Wv[:, h * HF : (h + 1) * HF]) for h in range(2)]
    wo_h = [np.ascontiguousarray(_bf16(Wo[h * HF : (h + 1) * HF, :])) for h in range(2)]
    bq_h = [np.ascontiguousarray(bq[h * HF : (h + 1) * HF]) for h in range(2)]
    bv_h = [
        np.ascontiguousarray(np.broadcast_to(bv[h * HF : (h + 1) * HF], (P, HF)))
        for h in range(2)
    ]

    nc = _get_program()
    in_maps = []
    for c in range(8):
        b, hh = divmod(c, 2)
        in_maps.append(
            {
                "x": x_b[b],
                "wq": wq_h[hh], "wk": wk_h[hh], "wv": wv_h[hh],
                "wo": wo_h[hh], "bq": bq_h[hh], "bv_b": bv_h[hh],
            }
        )

    kw = {}
    if _trace:
        kw = dict(trace=True, **(_trace_kwargs or {}))
    res = run_bass_kernel_spmd(nc, in_maps, list(range(8)), **kw)
    _CACHE["last_result"] = res

    outp = np.empty((4, T, C), np.float32)
    for b in range(4):
        p0 = res.results[2 * b]["out"].reshape(T, C)
        p1 = res.results[2 * b + 1]["out"].reshape(T, C)
        outp[b] = p0 + p1
    outp += bo.astype(np.float32)
    return outp


# revision 49
# speedup vs baseline: 1.0051x; 1.0034x over previous
"""Multi-head self-attention (B=4, T=2048, C=1024, H=16) on 8 Trainium2 cores.

Sharding (head-split): core c handles batch b = c//2 and head-half
hh = c%2 (8 of the 16 heads), ALL 2048 queries and keys of its batch.
No K/V projection redundancy. The output projection contracts only this
core's 512 feature columns, so each core returns a PARTIAL [2048, 1024]
fp32 product; the host sums the two partials per batch and adds bo.

Engine plan (measured: bf16 N=512 matmul back-to-back at 216 ns with
LDWEIGHTS hidden; K=64 matmul pairs at tile_position (0,0)/(64,0) run
CONCURRENTLY; ScalarE ACTIVATE = (N+352)/1.2 ns, dtype-independent):
  - ScalarE exp() of the 33.5M logits is the pacer: 256 x [128,1024]
    ACTIVATEs ~ 294 us.
  - PE: V projection upfront; K^T/Q^T of head pair hp+1 and the output
    projection of hp-1 are INTERLEAVED into hp's attention inner loop so
    the PE never idles long enough for the HAM activity monitor to
    re-throttle the clock, and no separate projection phases remain.
  - DVE: bias adds, PSUM->SBUF casts, softmax normalize.

Layouts are feature-on-partition throughout: X^T via DMA transpose (sync
queue ONLY - transpose on the Activation queue loses the completion
ordering and races); K^T/Q^T per head pair (2x64 features on partitions
0:63/64:127); V as [key-chunk, head, 64+ones] so softmax denominators
ride along row 64 of the AV accumulation.
"""
import sys

sys.path.insert(0, "/opt/trn_rl_repo")

from contextlib import ExitStack

import numpy as np

import concourse.bacc as bacc
import concourse.tile as tile
from concourse import library_config, mybir
from concourse.bass_utils import run_bass_kernel_spmd

F32 = mybir.dt.float32
BF16 = mybir.dt.bfloat16
AF = mybir.ActivationFunctionType

T, C, NH, D = 2048, 1024, 16, 64
HH = 8                  # heads per core
HF = HH * D             # 512 feature columns per core
P = 128
N_KC = C // P           # 8 contraction chunks
N_TT = T // P           # 16 token/key chunks
N_HP = HH // 2          # 4 head pairs per core
N_QP = 4                # query passes of 512
QW = T // N_QP          # 512 queries per pass
VW = D + 1              # per-head V width incl. ones column

_CACHE = {}


def _build(debug=False):
    nc = bacc.Bacc("TRN2", target_bir_lowering=False, debug=False)

    x = nc.declare_dram_parameter("x", [C, T], BF16, isOutput=False)  # X^T
    wq = nc.declare_dram_parameter("wq", [C, HF], BF16, isOutput=False)
    wk = nc.declare_dram_parameter("wk", [C, HF], BF16, isOutput=False)
    wv = nc.declare_dram_parameter("wv", [C, HF], BF16, isOutput=False)
    wo = nc.declare_dram_parameter("wo", [HF, C], BF16, isOutput=False)
    bq = nc.declare_dram_parameter("bq", [HF], F32, isOutput=False)
    bv_b = nc.declare_dram_parameter("bv_b", [P, HF], F32, isOutput=False)
    out = nc.declare_dram_parameter("out", [N_TT, P, C], F32, isOutput=True)

    dbg = {}
    if debug:
        for name, shape, dt_ in [
            ("dbg_xt", [P, T], BF16),
            ("dbg_qt", [P, T], BF16),
            ("dbg_kt", [P, T], BF16),
            ("dbg_vd", [P, HH * VW], BF16),
            ("dbg_s", [P, 2 * QW], F32),
            ("dbg_pt", [P, 2 * QW], BF16),
            ("dbg_o", [VW, QW], F32),
            ("dbg_rc", [1, QW], F32),
            ("dbg_bc", [64, QW], F32),
            ("dbg_ao", [P, T], BF16),
        ]:
            dbg[name] = nc.declare_dram_parameter(name, shape, dt_, isOutput=True)

    with tile.TileContext(nc) as tc, ExitStack() as ctx:
        big = ctx.enter_context(tc.tile_pool(name="big", bufs=1))
        pt_pool = ctx.enter_context(tc.tile_pool(name="pt", bufs=3))
        rc_pool = ctx.enter_context(tc.tile_pool(name="rc", bufs=2))
        bc_pool = ctx.enter_context(tc.tile_pool(name="bc", bufs=2))
        s_ps = ctx.enter_context(tc.tile_pool(name="sps", bufs=2, space="PSUM"))
        o_ps = ctx.enter_context(tc.tile_pool(name="ops", bufs=3, space="PSUM"))
        pr_ps = ctx.enter_context(tc.tile_pool(name="prps", bufs=1, space="PSUM"))

        nc.gpsimd.load_library(library_config.attn)

        # ---- inputs to SBUF -------------------------------------------------
        # DMA ordering is the startup critical path: X^T (pre-transposed on
        # host) and Wv gate the V projection. X^T lands token-slice-major so
        # V(tt=0) unblocks after ~0.5 MB instead of the full 4 MB.
        xt = big.tile([P, N_KC, T], BF16)          # X^T (c, t)
        qdma3 = [nc.sync, nc.scalar, nc.gpsimd]
        wv_t = big.tile([P, N_KC, HF], BF16)
        wk_t = big.tile([P, N_KC, HF], BF16)
        wq_t = big.tile([P, N_KC, HF], BF16)
        wdma = [nc.scalar, nc.gpsimd]
        # interleave Wv and X^T chunk loads so the V projection's per-kc
        # (xt, wv) pairs land together instead of xt queueing behind all of wv
        for kc in range(N_KC):
            wdma[kc % 2].dma_start(out=wv_t[:, kc, :], in_=wv[kc * P : (kc + 1) * P, :])
            qdma3[kc % 3].dma_start(
                out=xt[:, kc, :], in_=x[kc * P : (kc + 1) * P, :]
            )
        for kc in range(N_KC):
            wdma[kc % 2].dma_start(out=wk_t[:, kc, :], in_=wk[kc * P : (kc + 1) * P, :])
        for kc in range(N_KC):
            wdma[kc % 2].dma_start(out=wq_t[:, kc, :], in_=wq[kc * P : (kc + 1) * P, :])

        bq_t = big.tile([P, N_HP], F32)
        for hp in range(N_HP):
            nc.sync.dma_start(
                out=bq_t[:, hp : hp + 1], in_=bq[hp * P : (hp + 1) * P].unsqueeze(-1)
            )
        bv_t = big.tile([P, HF], F32)
        nc.sync.dma_start(out=bv_t[:, :], in_=bv_b[:, :])

        wo_t = big.tile([P, N_HP, C], BF16)
        for hp in range(N_HP):
            wdma[hp % 2].dma_start(out=wo_t[:, hp, :], in_=wo[hp * P : (hp + 1) * P, :])

        v_res = big.tile([P, N_TT, HH * VW], BF16)  # [v_h | 1] per head per chunk
        kt_res = big.tile([P, N_HP, T], BF16)       # K^T (f, t)
        qt = big.tile([P, N_HP, T], BF16)           # Q^T (f, q)
        attout = big.tile([P, N_HP, T], BF16)       # normalized O^T

        v_ones = v_res.rearrange("p t (h w) -> p t h w", w=VW)
        nc.vector.memset(v_ones[:, :, :, D : D + 1], 1.0)

        # ---- V = X @ Wv + bv, all heads (tokens on partitions) --------------
        bv_v = bv_t.rearrange("p (h d) -> p h d", h=HH)
        for tt in range(N_TT):
            pvf = s_ps.tile([P, 2 * QW], F32, tag="s")
            pv = pvf[:, 0:HF]
            for kc in range(N_KC):
                nc.tensor.matmul(
                    pv[:, :],
                    xt[:, kc, tt * P : (tt + 1) * P],
                    wv_t[:, kc, :],
                    start=(kc == 0),
                    stop=(kc == N_KC - 1),
                )
            pv_v = pv.rearrange("p (h d) -> p h d", h=HH)
            nc.vector.tensor_add(v_ones[:, tt, :, 0:D], pv_v[:, :, :], bv_v[:, :, :])

        # ---- projection work generators (emitted inline with attention) ----
        def k_proj_steps(hp, pool=None, tag="pr"):
            """K^T(hp): 4 th-groups x (8 accumulating MMs + a DVE cast)."""
            for th in range(N_QP):
                pk = (pool or pr_ps).tile([P, QW], F32, tag=tag)
                for kc in range(N_KC):
                    yield lambda hp=hp, th=th, kc=kc, pk=pk: nc.tensor.matmul(
                        pk[:, :],
                        wk_t[:, kc, hp * P : (hp + 1) * P],
                        xt[:, kc, th * QW : (th + 1) * QW],
                        start=(kc == 0),
                        stop=(kc == N_KC - 1),
                    )
                yield lambda hp=hp, th=th, pk=pk: nc.vector.tensor_copy(
                    kt_res[:, hp, th * QW : (th + 1) * QW], pk[:, :]
                )

        def q_proj_steps(hp, pool=None, tag="pr"):
            for th in range(N_QP):
                pq = (pool or pr_ps).tile([P, QW], F32, tag=tag)
                for kc in range(N_KC):
                    yield lambda hp=hp, th=th, kc=kc, pq=pq: nc.tensor.matmul(
                        pq[:, :],
                        wq_t[:, kc, hp * P : (hp + 1) * P],
                        xt[:, kc, th * QW : (th + 1) * QW],
                        start=(kc == 0),
                        stop=(kc == N_KC - 1),
                    )
                yield lambda hp=hp, th=th, pq=pq: nc.vector.tensor_scalar_add(
                    qt[:, hp, th * QW : (th + 1) * QW], pq[:, :], bq_t[:, hp : hp + 1]
                )

        odma = [nc.sync, nc.scalar, nc.gpsimd]

        def out_proj_steps(qms, use_s_pool=False):
            """Output projection for query chunks qms (contract all 4 hp)."""
            for qm in qms:
                for nh in range(2):
                    if use_s_pool and (qm + nh) % 2 == 0:
                        # tail only: the s pool is idle after the last ACT
                        po_f = s_ps.tile([P, 2 * QW], F32, tag="s")
                        po = po_f[:, 0:QW]
                    else:
                        po = pr_ps.tile([P, QW], F32, tag="pr")
                    for hp in range(N_HP):
                        yield lambda qm=qm, nh=nh, hp=hp, po=po: nc.tensor.matmul(
                            po[:, :],
                            attout[:, hp, qm * P : (qm + 1) * P],
                            wo_t[:, hp, nh * QW : (nh + 1) * QW],
                            start=(hp == 0),
                            stop=(hp == N_HP - 1),
                        )

                    def _drain(qm=qm, nh=nh, po=po):
                        os_ = bc_pool.tile([P, QW], F32, tag="os")
                        nc.vector.tensor_copy(os_[:, :], po[:, :])
                        odma[(2 * qm + nh) % 3].dma_start(
                            out=out[qm, :, nh * QW : (nh + 1) * QW], in_=os_[:, :]
                        )

                    yield _drain

        def chain(*gens):
            for g in gens:
                yield from g

        # upfront: K^T(0), Q^T(0) (V is already queued above); these use
        # the s pool (idle until attention starts)

        def _s_pool_qw():
            class p:
                @staticmethod
                def tile(shape, dt_, tag=None):
                    t = s_ps.tile([P, 2 * QW], dt_, tag="s")
                    return t[:, 0 : shape[1]]
            return p

        for step in chain(
            k_proj_steps(0, pool=_s_pool_qw()), q_proj_steps(0, pool=_s_pool_qw())
        ):
            step()

        # side work emitted during attention inner loops. The chip power
        # manager allows ~160-200 us of full-rate PE, then duty-cycles the
        # clock to ~0.686 - so ALL projection side work is front-loaded into
        # the first two head pairs (the grace window), leaving hp 2..3 pure
        # attention (~0.69 PE duty, which the clamp tolerates at full pace).
        # Out-proj needs ALL head pairs' attout, so it can only run during
        # hp 3 (pass qp covers chunks of pass qp-1) plus a tail.
        side = {}
        for hp in range(3):
            g = chain(k_proj_steps(hp + 1), q_proj_steps(hp + 1))
            for qp in range(N_QP):
                side[(hp, qp)] = (g, 72 / 64)
        side[(3, 0)] = (iter(()), 0.0)
        for qp in range(1, N_QP):
            side[(3, qp)] = (out_proj_steps(range(4 * (qp - 1), 4 * qp)), 36 / 16)

        # ---- attention: per head pair, per query pass -----------------------
        for hp in range(N_HP):
            hA, hB = 2 * hp, 2 * hp + 1
            for qp in range(N_QP):
                gen, side_per_iter = side[(hp, qp)]
                quota = 0.0
                q0 = qp * QW
                oA = o_ps.tile([VW, QW], F32, tag="o")
                oB = o_ps.tile([VW, QW], F32, tag="o")
                for kt in range(N_TT):
                    s = s_ps.tile([P, 2 * QW], F32, tag="s")
                    nc.tensor.matmul(
                        s[:, 0:QW],
                        kt_res[0:64, hp, kt * P : (kt + 1) * P],
                        qt[0:64, hp, q0 : q0 + QW],
                        start=True,
                        stop=True,
                        tile_position=(0, 0),
                    )
                    nc.tensor.matmul(
                        s[:, QW : 2 * QW],
                        kt_res[64:128, hp, kt * P : (kt + 1) * P],
                        qt[64:128, hp, q0 : q0 + QW],
                        start=True,
                        stop=True,
                        tile_position=(64, 0),
                    )
                    p_t = pt_pool.tile([P, 2 * QW], BF16, tag="pt")
                    nc.scalar.activation(p_t[:, :], s[:, :], AF.Exp, scale=0.125)
                    if debug and hp == 0 and qp == 0 and kt == 0:
                        dcp = bc_pool.tile([P, 2 * QW], F32, tag="dbgs")
                        nc.vector.tensor_copy(dcp[:, :], s[:, :])
                        nc.sync.dma_start(out=dbg["dbg_s"][:, :], in_=dcp[:, :])
                        nc.sync.dma_start(out=dbg["dbg_pt"][:, :], in_=p_t[:, :])
                    # emit interleaved projection work here, between the S
                    # pair and the AV accumulation: the AVs wait on the ACT
                    # anyway, so side MMs slot into that PE gap instead of
                    # queueing behind the AVs and blocking the next S pair
                    quota += side_per_iter
                    while quota >= 1.0:
                        step = next(gen, None)
                        if step is None:
                            quota = 0.0
                            break
                        step()
                        quota -= 1.0
                    nc.tensor.matmul(
                        oA[:, :],
                        v_res[:, kt, hA * VW : (hA + 1) * VW],
                        p_t[:, 0:QW],
                        start=(kt == 0),
                        stop=(kt == N_TT - 1),
                    )
                    nc.tensor.matmul(
                        oB[:, :],
                        v_res[:, kt, hB * VW : (hB + 1) * VW],
                        p_t[:, QW : 2 * QW],
                        start=(kt == 0),
                        stop=(kt == N_TT - 1),
                    )

                if debug and hp == 0 and qp == 0:
                    ocp = bc_pool.tile([VW, QW], F32, tag="dbgo")
                    nc.vector.tensor_copy(ocp[:, :], oA[:, :])
                    nc.sync.dma_start(out=dbg["dbg_o"][:, :], in_=ocp[:, :])
                # normalize: attout[d, q] = O[d, q] / O[64, q].
                # Copy O and den out of PSUM FIRST so the O banks free after
                # ~1.2us (the next pass's AV accumulation reuses them); the
                # recip -> gpsimd-broadcast -> mul chain then runs off the
                # PSUM critical path entirely.
                chains = []
                for row0, o_t in ((0, oA), (64, oB)):
                    o_sb = bc_pool.tile([64, QW], F32, tag="osb")
                    nc.vector.tensor_copy(o_sb[:, :], o_t[0:64, :])
                    den_t = rc_pool.tile([1, QW], F32, tag="den")
                    nc.vector.tensor_copy(den_t[:, :], o_t[64:VW, :])
                    chains.append((row0, o_sb, den_t))
                for row0, o_sb, den_t in chains:
                    rc_t = rc_pool.tile([1, QW], F32, tag="rc")
                    nc.vector.reciprocal_approx_fast(out=rc_t[:, :], in_=den_t[:, :])
                    bc_t = bc_pool.tile([64, QW], F32, tag="bc")
                    nc.gpsimd.partition_broadcast(bc_t[:, :], rc_t[:, :])
                    nc.vector.tensor_mul(
                        attout[row0 : row0 + 64, hp, q0 : q0 + QW],
                        o_sb[:, :],
                        bc_t[:, :],
                    )
                    if debug and hp == 0 and qp == 0 and row0 == 0:
                        nc.sync.dma_start(out=dbg["dbg_rc"][:, :], in_=rc_t[:, :])
                        nc.sync.dma_start(out=dbg["dbg_bc"][:, :], in_=bc_t[:, :])
                # drain leftover side work (shared gens span all 4 passes)
                if qp == N_QP - 1 or hp == 3:
                    for step in gen:
                        step()

        if debug:
            nc.sync.dma_start(out=dbg["dbg_xt"][:, :], in_=xt[:, 0, :])
            nc.sync.dma_start(out=dbg["dbg_qt"][:, :], in_=qt[:, 0, :])
            nc.sync.dma_start(out=dbg["dbg_kt"][:, :], in_=kt_res[:, 0, :])
            nc.sync.dma_start(out=dbg["dbg_vd"][:, :], in_=v_res[:, 0, :])
            nc.sync.dma_start(out=dbg["dbg_ao"][:, :], in_=attout[:, 0, :])

        # ---- output projection tail (qm 12..15; rest ran inside hp 3) -------
        for step in out_proj_steps(range(12, N_TT), use_s_pool=True):
            step()

    nc.finalize()
    return nc


def _get_program():
    if "nc" not in _CACHE:
        _CACHE["nc"] = _build()
    return _CACHE["nc"]


def _bf16(a):
    import ml_dtypes

    return np.asarray(a, np.float32).astype(ml_dtypes.bfloat16)


def kernel(x, Wq, bq, Wk, bk, Wv, bv, Wo, bo, _trace=False, _trace_kwargs=None):
    x = np.asarray(x, np.float32)
    bq, bv, bo = (np.asarray(b, np.float32) for b in (bq, bv, bo))
    # bk unused: a key-side bias adds a per-query constant to every logit of a
    # softmax row, which cancels exactly in the softmax.

    x_b = [np.ascontiguousarray(_bf16(x[b]).T) for b in range(4)]
    wq_h = [_bf16(Wq[:, h * HF : (h + 1) * HF]) for h in range(2)]
    wk_h = [_bf16(Wk[:, h * HF : (h + 1) * HF]) for h in range(2)]
    wv_h = [_bf16(Wv[:, h * HF : (h + 1) * HF]) for h in range(2)]
    wo_h = [np.ascontiguousarray(_bf16(Wo[h * HF : (h + 1) * HF, :])) for h in range(2)]
    bq_h = [np.ascontiguousarray(bq[h * HF : (h + 1) * HF]) for h in range(2)]
    bv_h = [
        np.ascontiguousarray(np.broadcast_to(bv[h * HF : (h + 1) * HF], (P, HF)))
        for h in range(2)
    ]

    nc = _get_program()
    in_maps = []
    for c in range(8):
        b, hh = divmod(c, 2)
        in_maps.append(
            {
                "x": x_b[b],
                "wq": wq_h[hh], "wk": wk_h[hh], "wv": wv_h[hh],
                "wo": wo_h[hh], "bq": bq_h[hh], "bv_b": bv_h[hh],
            }
        )

    kw = {}
    if _trace:
        kw = dict(trace=True, **(_trace_kwargs or {}))
    res = run_bass_kernel_spmd(nc, in_maps, list(range(8)), **kw)
    _CACHE["last_result"] = res

    outp = np.empty((4, T, C), np.float32)
    for b in range(4):
        p0 = res.results[2 * b]["out"].reshape(T, C)
        p1 = res.results[2 * b + 1]["out"].reshape(T, C)
        outp[b] = p0 + p1
    outp += bo.astype(np.float32)
    return outp


# revision 50
# speedup vs baseline: 1.0061x; 1.0010x over previous
"""Multi-head self-attention (B=4, T=2048, C=1024, H=16) on 8 Trainium2 cores.

Sharding (head-split): core c handles batch b = c//2 and head-half
hh = c%2 (8 of the 16 heads), ALL 2048 queries and keys of its batch.
No K/V projection redundancy. The output projection contracts only this
core's 512 feature columns, so each core returns a PARTIAL [2048, 1024]
fp32 product; the host sums the two partials per batch and adds bo.

Engine plan (measured: bf16 N=512 matmul back-to-back at 216 ns with
LDWEIGHTS hidden; K=64 matmul pairs at tile_position (0,0)/(64,0) run
CONCURRENTLY; ScalarE ACTIVATE = (N+352)/1.2 ns, dtype-independent):
  - ScalarE exp() of the 33.5M logits is the pacer: 256 x [128,1024]
    ACTIVATEs ~ 294 us.
  - PE: V projection upfront; K^T/Q^T of head pair hp+1 and the output
    projection of hp-1 are INTERLEAVED into hp's attention inner loop so
    the PE never idles long enough for the HAM activity monitor to
    re-throttle the clock, and no separate projection phases remain.
  - DVE: bias adds, PSUM->SBUF casts, softmax normalize.

Layouts are feature-on-partition throughout: X^T via DMA transpose (sync
queue ONLY - transpose on the Activation queue loses the completion
ordering and races); K^T/Q^T per head pair (2x64 features on partitions
0:63/64:127); V as [key-chunk, head, 64+ones] so softmax denominators
ride along row 64 of the AV accumulation.
"""
import sys

sys.path.insert(0, "/opt/trn_rl_repo")

from contextlib import ExitStack

import numpy as np

import concourse.bacc as bacc
import concourse.tile as tile
from concourse import library_config, mybir
from concourse.bass_utils import run_bass_kernel_spmd

F32 = mybir.dt.float32
BF16 = mybir.dt.bfloat16
AF = mybir.ActivationFunctionType

T, C, NH, D = 2048, 1024, 16, 64
HH = 8                  # heads per core
HF = HH * D             # 512 feature columns per core
P = 128
N_KC = C // P           # 8 contraction chunks
N_TT = T // P           # 16 token/key chunks
N_HP = HH // 2          # 4 head pairs per core
N_QP = 4                # query passes of 512
QW = T // N_QP          # 512 queries per pass
VW = D + 1              # per-head V width incl. ones column

_CACHE = {}


def _build(debug=False):
    nc = bacc.Bacc("TRN2", target_bir_lowering=False, debug=False)

    x = nc.declare_dram_parameter("x", [C, T], BF16, isOutput=False)  # X^T
    wq = nc.declare_dram_parameter("wq", [C, HF], BF16, isOutput=False)
    wk = nc.declare_dram_parameter("wk", [C, HF], BF16, isOutput=False)
    wv = nc.declare_dram_parameter("wv", [C, HF], BF16, isOutput=False)
    wo = nc.declare_dram_parameter("wo", [HF, C], BF16, isOutput=False)
    bq = nc.declare_dram_parameter("bq", [HF], F32, isOutput=False)
    bv_b = nc.declare_dram_parameter("bv_b", [P, HF], F32, isOutput=False)
    out = nc.declare_dram_parameter("out", [N_TT, P, C], F32, isOutput=True)

    dbg = {}
    if debug:
        for name, shape, dt_ in [
            ("dbg_xt", [P, T], BF16),
            ("dbg_qt", [P, T], BF16),
            ("dbg_kt", [P, T], BF16),
            ("dbg_vd", [P, HH * VW], BF16),
            ("dbg_s", [P, 2 * QW], F32),
            ("dbg_pt", [P, 2 * QW], BF16),
            ("dbg_o", [VW, QW], F32),
            ("dbg_rc", [1, QW], F32),
            ("dbg_bc", [64, QW], F32),
            ("dbg_ao", [P, T], BF16),
        ]:
            dbg[name] = nc.declare_dram_parameter(name, shape, dt_, isOutput=True)

    with tile.TileContext(nc) as tc, ExitStack() as ctx:
        big = ctx.enter_context(tc.tile_pool(name="big", bufs=1))
        pt_pool = ctx.enter_context(tc.tile_pool(name="pt", bufs=3))
        rc_pool = ctx.enter_context(tc.tile_pool(name="rc", bufs=2))
        bc_pool = ctx.enter_context(tc.tile_pool(name="bc", bufs=2))
        s_ps = ctx.enter_context(tc.tile_pool(name="sps", bufs=2, space="PSUM"))
        o_ps = ctx.enter_context(tc.tile_pool(name="ops", bufs=3, space="PSUM"))
        pr_ps = ctx.enter_context(tc.tile_pool(name="prps", bufs=1, space="PSUM"))

        nc.gpsimd.load_library(library_config.attn)

        # ---- inputs to SBUF -------------------------------------------------
        # DMA ordering is the startup critical path: X^T (pre-transposed on
        # host) and Wv gate the V projection. X^T lands token-slice-major so
        # V(tt=0) unblocks after ~0.5 MB instead of the full 4 MB.
        xt = big.tile([P, N_KC, T], BF16)          # X^T (c, t)
        qdma3 = [nc.sync, nc.scalar, nc.gpsimd]
        wv_t = big.tile([P, N_KC, HF], BF16)
        wk_t = big.tile([P, N_KC, HF], BF16)
        wq_t = big.tile([P, N_KC, HF], BF16)
        wdma = [nc.scalar, nc.gpsimd]
        # interleave Wv and X^T chunk loads so the V projection's per-kc
        # (xt, wv) pairs land together instead of xt queueing behind all of wv
        for kc in range(N_KC):
            wdma[kc % 2].dma_start(out=wv_t[:, kc, :], in_=wv[kc * P : (kc + 1) * P, :])
            qdma3[kc % 3].dma_start(
                out=xt[:, kc, :], in_=x[kc * P : (kc + 1) * P, :]
            )
        for kc in range(N_KC):
            wdma[kc % 2].dma_start(out=wk_t[:, kc, :], in_=wk[kc * P : (kc + 1) * P, :])
        for kc in range(N_KC):
            wdma[kc % 2].dma_start(out=wq_t[:, kc, :], in_=wq[kc * P : (kc + 1) * P, :])

        bq_t = big.tile([P, N_HP], F32)
        for hp in range(N_HP):
            nc.sync.dma_start(
                out=bq_t[:, hp : hp + 1], in_=bq[hp * P : (hp + 1) * P].unsqueeze(-1)
            )
        bv_t = big.tile([P, HF], F32)
        nc.sync.dma_start(out=bv_t[:, :], in_=bv_b[:, :])

        wo_t = big.tile([P, N_HP, C], BF16)
        for hp in range(N_HP):
            wdma[hp % 2].dma_start(out=wo_t[:, hp, :], in_=wo[hp * P : (hp + 1) * P, :])

        v_res = big.tile([P, N_TT, HH * VW], BF16)  # [v_h | 1] per head per chunk
        kt_res = big.tile([P, N_HP, T], BF16)       # K^T (f, t)
        qt = big.tile([P, N_HP, T], BF16)           # Q^T (f, q)
        attout = big.tile([P, N_HP, T], BF16)       # normalized O^T

        v_ones = v_res.rearrange("p t (h w) -> p t h w", w=VW)
        nc.vector.memset(v_ones[:, :, :, D : D + 1], 1.0)

        # ---- V = X @ Wv + bv, all heads (tokens on partitions) --------------
        bv_v = bv_t.rearrange("p (h d) -> p h d", h=HH)
        for tt in range(N_TT):
            if tt % 3 < 2:
                pvf = s_ps.tile([P, 2 * QW], F32, tag="s")
                pv = pvf[:, 0:HF]
            else:
                pv = pr_ps.tile([P, HF], F32, tag="pr")
            for kc in range(N_KC):
                nc.tensor.matmul(
                    pv[:, :],
                    xt[:, kc, tt * P : (tt + 1) * P],
                    wv_t[:, kc, :],
                    start=(kc == 0),
                    stop=(kc == N_KC - 1),
                )
            pv_v = pv.rearrange("p (h d) -> p h d", h=HH)
            nc.vector.tensor_add(v_ones[:, tt, :, 0:D], pv_v[:, :, :], bv_v[:, :, :])

        # ---- projection work generators (emitted inline with attention) ----
        def k_proj_steps(hp, pool=None, tag="pr"):
            """K^T(hp): 4 th-groups x (8 accumulating MMs + a DVE cast)."""
            for th in range(N_QP):
                pk = (pool or pr_ps).tile([P, QW], F32, tag=tag)
                for kc in range(N_KC):
                    yield lambda hp=hp, th=th, kc=kc, pk=pk: nc.tensor.matmul(
                        pk[:, :],
                        wk_t[:, kc, hp * P : (hp + 1) * P],
                        xt[:, kc, th * QW : (th + 1) * QW],
                        start=(kc == 0),
                        stop=(kc == N_KC - 1),
                    )
                yield lambda hp=hp, th=th, pk=pk: nc.vector.tensor_copy(
                    kt_res[:, hp, th * QW : (th + 1) * QW], pk[:, :]
                )

        def q_proj_steps(hp, pool=None, tag="pr"):
            for th in range(N_QP):
                pq = (pool or pr_ps).tile([P, QW], F32, tag=tag)
                for kc in range(N_KC):
                    yield lambda hp=hp, th=th, kc=kc, pq=pq: nc.tensor.matmul(
                        pq[:, :],
                        wq_t[:, kc, hp * P : (hp + 1) * P],
                        xt[:, kc, th * QW : (th + 1) * QW],
                        start=(kc == 0),
                        stop=(kc == N_KC - 1),
                    )
                yield lambda hp=hp, th=th, pq=pq: nc.vector.tensor_scalar_add(
                    qt[:, hp, th * QW : (th + 1) * QW], pq[:, :], bq_t[:, hp : hp + 1]
                )

        odma = [nc.sync, nc.scalar, nc.gpsimd]

        def out_proj_steps(qms, use_s_pool=False):
            """Output projection for query chunks qms (contract all 4 hp)."""
            for qm in qms:
                for nh in range(2):
                    if use_s_pool and (qm + nh) % 2 == 0:
                        # tail only: the s pool is idle after the last ACT
                        po_f = s_ps.tile([P, 2 * QW], F32, tag="s")
                        po = po_f[:, 0:QW]
                    else:
                        po = pr_ps.tile([P, QW], F32, tag="pr")
                    for hp in range(N_HP):
                        yield lambda qm=qm, nh=nh, hp=hp, po=po: nc.tensor.matmul(
                            po[:, :],
                            attout[:, hp, qm * P : (qm + 1) * P],
                            wo_t[:, hp, nh * QW : (nh + 1) * QW],
                            start=(hp == 0),
                            stop=(hp == N_HP - 1),
                        )

                    def _drain(qm=qm, nh=nh, po=po):
                        os_ = bc_pool.tile([P, QW], F32, tag="os")
                        nc.vector.tensor_copy(os_[:, :], po[:, :])
                        odma[(2 * qm + nh) % 3].dma_start(
                            out=out[qm, :, nh * QW : (nh + 1) * QW], in_=os_[:, :]
                        )

                    yield _drain

        def chain(*gens):
            for g in gens:
                yield from g

        # upfront: K^T(0), Q^T(0) (V is already queued above); these use
        # the s pool (idle until attention starts)

        def _s_pool_qw():
            class p:
                @staticmethod
                def tile(shape, dt_, tag=None):
                    t = s_ps.tile([P, 2 * QW], dt_, tag="s")
                    return t[:, 0 : shape[1]]
            return p

        for step in chain(
            k_proj_steps(0, pool=_s_pool_qw()), q_proj_steps(0, pool=_s_pool_qw())
        ):
            step()

        # side work emitted during attention inner loops. The chip power
        # manager allows ~160-200 us of full-rate PE, then duty-cycles the
        # clock to ~0.686 - so ALL projection side work is front-loaded into
        # the first two head pairs (the grace window), leaving hp 2..3 pure
        # attention (~0.69 PE duty, which the clamp tolerates at full pace).
        # Out-proj needs ALL head pairs' attout, so it can only run during
        # hp 3 (pass qp covers chunks of pass qp-1) plus a tail.
        side = {}
        for hp in range(3):
            g = chain(k_proj_steps(hp + 1), q_proj_steps(hp + 1))
            for qp in range(N_QP):
                side[(hp, qp)] = (g, 72 / 56)
        side[(3, 0)] = (iter(()), 0.0)
        for qp in range(1, N_QP):
            side[(3, qp)] = (out_proj_steps(range(4 * (qp - 1), 4 * qp)), 36 / 14)

        # ---- attention: per head pair, per query pass -----------------------
        for hp in range(N_HP):
            hA, hB = 2 * hp, 2 * hp + 1
            for qp in range(N_QP):
                gen, side_per_iter = side[(hp, qp)]
                quota = 0.0
                q0 = qp * QW
                oA = o_ps.tile([VW, QW], F32, tag="o")
                oB = o_ps.tile([VW, QW], F32, tag="o")
                for kt in range(N_TT):
                    s = s_ps.tile([P, 2 * QW], F32, tag="s")
                    nc.tensor.matmul(
                        s[:, 0:QW],
                        kt_res[0:64, hp, kt * P : (kt + 1) * P],
                        qt[0:64, hp, q0 : q0 + QW],
                        start=True,
                        stop=True,
                        tile_position=(0, 0),
                    )
                    nc.tensor.matmul(
                        s[:, QW : 2 * QW],
                        kt_res[64:128, hp, kt * P : (kt + 1) * P],
                        qt[64:128, hp, q0 : q0 + QW],
                        start=True,
                        stop=True,
                        tile_position=(64, 0),
                    )
                    p_t = pt_pool.tile([P, 2 * QW], BF16, tag="pt")
                    nc.scalar.activation(p_t[:, :], s[:, :], AF.Exp, scale=0.125)
                    if debug and hp == 0 and qp == 0 and kt == 0:
                        dcp = bc_pool.tile([P, 2 * QW], F32, tag="dbgs")
                        nc.vector.tensor_copy(dcp[:, :], s[:, :])
                        nc.sync.dma_start(out=dbg["dbg_s"][:, :], in_=dcp[:, :])
                        nc.sync.dma_start(out=dbg["dbg_pt"][:, :], in_=p_t[:, :])
                    # emit interleaved projection work here, between the S
                    # pair and the AV accumulation: the AVs wait on the ACT
                    # anyway, so side MMs slot into that PE gap instead of
                    # queueing behind the AVs and blocking the next S pair
                    if (kt >= 2) if hp == 3 else (kt < 14):
                        quota += side_per_iter
                    while quota >= 1.0:
                        step = next(gen, None)
                        if step is None:
                            quota = 0.0
                            break
                        step()
                        quota -= 1.0
                    nc.tensor.matmul(
                        oA[:, :],
                        v_res[:, kt, hA * VW : (hA + 1) * VW],
                        p_t[:, 0:QW],
                        start=(kt == 0),
                        stop=(kt == N_TT - 1),
                    )
                    nc.tensor.matmul(
                        oB[:, :],
                        v_res[:, kt, hB * VW : (hB + 1) * VW],
                        p_t[:, QW : 2 * QW],
                        start=(kt == 0),
                        stop=(kt == N_TT - 1),
                    )

                if debug and hp == 0 and qp == 0:
                    ocp = bc_pool.tile([VW, QW], F32, tag="dbgo")
                    nc.vector.tensor_copy(ocp[:, :], oA[:, :])
                    nc.sync.dma_start(out=dbg["dbg_o"][:, :], in_=ocp[:, :])
                # normalize: attout[d, q] = O[d, q] / O[64, q].
                # Copy O and den out of PSUM FIRST so the O banks free after
                # ~1.2us (the next pass's AV accumulation reuses them); the
                # recip -> gpsimd-broadcast -> mul chain then runs off the
                # PSUM critical path entirely.
                chains = []
                for row0, o_t in ((0, oA), (64, oB)):
                    o_sb = bc_pool.tile([64, QW], F32, tag="osb")
                    nc.vector.tensor_copy(o_sb[:, :], o_t[0:64, :])
                    den_t = rc_pool.tile([1, QW], F32, tag="den")
                    nc.vector.tensor_copy(den_t[:, :], o_t[64:VW, :])
                    chains.append((row0, o_sb, den_t))
                for row0, o_sb, den_t in chains:
                    rc_t = rc_pool.tile([1, QW], F32, tag="rc")
                    nc.vector.reciprocal_approx_fast(out=rc_t[:, :], in_=den_t[:, :])
                    bc_t = bc_pool.tile([64, QW], F32, tag="bc")
                    nc.gpsimd.partition_broadcast(bc_t[:, :], rc_t[:, :])
                    nc.vector.tensor_mul(
                        attout[row0 : row0 + 64, hp, q0 : q0 + QW],
                        o_sb[:, :],
                        bc_t[:, :],
                    )
                    if debug and hp == 0 and qp == 0 and row0 == 0:
                        nc.sync.dma_start(out=dbg["dbg_rc"][:, :], in_=rc_t[:, :])
                        nc.sync.dma_start(out=dbg["dbg_bc"][:, :], in_=bc_t[:, :])
                # drain leftover side work (shared gens span all 4 passes)
                if qp == N_QP - 1 or hp == 3:
                    for step in gen:
                        step()

        if debug:
            nc.sync.dma_start(out=dbg["dbg_xt"][:, :], in_=xt[:, 0, :])
            nc.sync.dma_start(out=dbg["dbg_qt"][:, :], in_=qt[:, 0, :])
            nc.sync.dma_start(out=dbg["dbg_kt"][:, :], in_=kt_res[:, 0, :])
            nc.sync.dma_start(out=dbg["dbg_vd"][:, :], in_=v_res[:, 0, :])
            nc.sync.dma_start(out=dbg["dbg_ao"][:, :], in_=attout[:, 0, :])

        # ---- output projection tail (qm 12..15; rest ran inside hp 3) -------
        for step in out_proj_steps(range(12, N_TT), use_s_pool=True):
            step()

    nc.finalize()
    return nc


def _get_program():
    if "nc" not in _CACHE:
        _CACHE["nc"] = _build()
    return _CACHE["nc"]


def _bf16(a):
    import ml_dtypes

    return np.asarray(a, np.float32).astype(ml_dtypes.bfloat16)


def kernel(x, Wq, bq, Wk, bk, Wv, bv, Wo, bo, _trace=False, _trace_kwargs=None):
    x = np.asarray(x, np.float32)
    bq, bv, bo = (np.asarray(b, np.float32) for b in (bq, bv, bo))
    # bk unused: a key-side bias adds a per-query constant to every logit of a
    # softmax row, which cancels exactly in the softmax.

    x_b = [np.ascontiguousarray(_bf16(x[b]).T) for b in range(4)]
    wq_h = [_bf16(Wq[:, h * HF : (h + 1) * HF]) for h in range(2)]
    wk_h = [_bf16(Wk[:, h * HF : (h + 1) * HF]) for h in range(2)]
    wv_h = [_bf16(Wv[:, h * HF : (h + 1) * HF]) for h in range(2)]
    wo_h = [np.ascontiguousarray(_bf16(Wo[h * HF : (h + 1) * HF, :])) for h in range(2)]
    bq_h = [np.ascontiguousarray(bq[h * HF : (h + 1) * HF]) for h in range(2)]
    bv_h = [
        np.ascontiguousarray(np.broadcast_to(bv[h * HF : (h + 1) * HF], (P, HF)))
        for h in range(2)
    ]

    nc = _get_program()
    in_maps = []
    for c in range(8):
        b, hh = divmod(c, 2)
        in_maps.append(
            {
                "x": x_b[b],
                "wq": wq_h[hh], "wk": wk_h[hh], "wv": wv_h[hh],
                "wo": wo_h[hh], "bq": bq_h[hh], "bv_b": bv_h[hh],
            }
        )

    kw = {}
    if _trace:
        kw = dict(trace=True, **(_trace_kwargs or {}))
    res = run_bass_kernel_spmd(nc, in_maps, list(range(8)), **kw)
    _CACHE["last_result"] = res

    outp = np.empty((4, T, C), np.float32)
    for b in range(4):
        p0 = res.results[2 * b]["out"].reshape(T, C)
        p1 = res.results[2 * b + 1]["out"].reshape(T, C)
        outp[b] = p0 + p1
    outp += bo.astype(np.float32)
    return outp


# revision 51
# speedup vs baseline: 1.0154x; 1.0092x over previous
"""Multi-head self-attention (B=4, T=2048, C=1024, H=16) on 8 Trainium2 cores.

Sharding (head-split): core c handles batch b = c//2 and head-half
hh = c%2 (8 of the 16 heads), ALL 2048 queries and keys of its batch.
No K/V projection redundancy. The output projection contracts only this
core's 512 feature columns, so each core returns a PARTIAL [2048, 1024]
fp32 product; the host sums the two partials per batch and adds bo.

Engine plan (measured: bf16 N=512 matmul back-to-back at 216 ns with
LDWEIGHTS hidden; K=64 matmul pairs at tile_position (0,0)/(64,0) run
CONCURRENTLY; ScalarE ACTIVATE = (N+352)/1.2 ns, dtype-independent):
  - ScalarE exp() of the 33.5M logits is the pacer: 256 x [128,1024]
    ACTIVATEs ~ 294 us.
  - PE: V projection upfront; K^T/Q^T of head pair hp+1 and the output
    projection of hp-1 are INTERLEAVED into hp's attention inner loop so
    the PE never idles long enough for the HAM activity monitor to
    re-throttle the clock, and no separate projection phases remain.
  - DVE: bias adds, PSUM->SBUF casts, softmax normalize.

Layouts are feature-on-partition throughout: X^T via DMA transpose (sync
queue ONLY - transpose on the Activation queue loses the completion
ordering and races); K^T/Q^T per head pair (2x64 features on partitions
0:63/64:127); V as [key-chunk, head, 64+ones] so softmax denominators
ride along row 64 of the AV accumulation.
"""
import sys

sys.path.insert(0, "/opt/trn_rl_repo")

from contextlib import ExitStack

import numpy as np

import concourse.bacc as bacc
import concourse.tile as tile
from concourse import library_config, mybir
from concourse.bass_utils import run_bass_kernel_spmd

F32 = mybir.dt.float32
BF16 = mybir.dt.bfloat16
AF = mybir.ActivationFunctionType

T, C, NH, D = 2048, 1024, 16, 64
HH = 8                  # heads per core
HF = HH * D             # 512 feature columns per core
P = 128
N_KC = C // P           # 8 contraction chunks
N_TT = T // P           # 16 token/key chunks
N_HP = HH // 2          # 4 head pairs per core
N_QP = 4                # query passes of 512
QW = T // N_QP          # 512 queries per pass
VW = D + 1              # per-head V width incl. ones column

_CACHE = {}


def _build(debug=False):
    nc = bacc.Bacc("TRN2", target_bir_lowering=False, debug=False)

    x = nc.declare_dram_parameter("x", [C, T], BF16, isOutput=False)  # X^T
    wq = nc.declare_dram_parameter("wq", [C, HF], BF16, isOutput=False)
    wk = nc.declare_dram_parameter("wk", [C, HF], BF16, isOutput=False)
    wv = nc.declare_dram_parameter("wv", [C, HF], BF16, isOutput=False)
    wo = nc.declare_dram_parameter("wo", [HF, C], BF16, isOutput=False)
    bq = nc.declare_dram_parameter("bq", [HF], F32, isOutput=False)
    bv_b = nc.declare_dram_parameter("bv_b", [P, HF], F32, isOutput=False)
    out = nc.declare_dram_parameter("out", [N_TT, P, C], F32, isOutput=True)

    dbg = {}
    if debug:
        for name, shape, dt_ in [
            ("dbg_xt", [P, T], BF16),
            ("dbg_qt", [P, T], BF16),
            ("dbg_kt", [P, T], BF16),
            ("dbg_vd", [P, HH * VW], BF16),
            ("dbg_s", [P, 2 * QW], F32),
            ("dbg_pt", [P, 2 * QW], BF16),
            ("dbg_o", [VW, QW], F32),
            ("dbg_rc", [1, QW], F32),
            ("dbg_bc", [64, QW], F32),
            ("dbg_ao", [P, T], BF16),
        ]:
            dbg[name] = nc.declare_dram_parameter(name, shape, dt_, isOutput=True)

    with tile.TileContext(nc) as tc, ExitStack() as ctx:
        big = ctx.enter_context(tc.tile_pool(name="big", bufs=1))
        pt_pool = ctx.enter_context(tc.tile_pool(name="pt", bufs=3))
        rc_pool = ctx.enter_context(tc.tile_pool(name="rc", bufs=2))
        bc_pool = ctx.enter_context(tc.tile_pool(name="bc", bufs=2))
        s_ps = ctx.enter_context(tc.tile_pool(name="sps", bufs=2, space="PSUM"))
        o_ps = ctx.enter_context(tc.tile_pool(name="ops", bufs=2, space="PSUM"))
        pr_ps = ctx.enter_context(tc.tile_pool(name="prps", bufs=2, space="PSUM"))

        nc.gpsimd.load_library(library_config.attn)

        # ---- inputs to SBUF -------------------------------------------------
        # DMA ordering is the startup critical path: X^T (pre-transposed on
        # host) and Wv gate the V projection. X^T lands token-slice-major so
        # V(tt=0) unblocks after ~0.5 MB instead of the full 4 MB.
        xt = big.tile([P, N_KC, T], BF16)          # X^T (c, t)
        qdma3 = [nc.sync, nc.scalar, nc.gpsimd]
        wv_t = big.tile([P, N_KC, HF], BF16)
        wk_t = big.tile([P, N_KC, HF], BF16)
        wq_t = big.tile([P, N_KC, HF], BF16)
        wdma = [nc.scalar, nc.gpsimd]
        # interleave Wv and X^T chunk loads so the V projection's per-kc
        # (xt, wv) pairs land together instead of xt queueing behind all of wv
        for kc in range(N_KC):
            wdma[kc % 2].dma_start(out=wv_t[:, kc, :], in_=wv[kc * P : (kc + 1) * P, :])
            qdma3[kc % 3].dma_start(
                out=xt[:, kc, :], in_=x[kc * P : (kc + 1) * P, :]
            )
        for kc in range(N_KC):
            wdma[kc % 2].dma_start(out=wk_t[:, kc, :], in_=wk[kc * P : (kc + 1) * P, :])
        for kc in range(N_KC):
            wdma[kc % 2].dma_start(out=wq_t[:, kc, :], in_=wq[kc * P : (kc + 1) * P, :])

        bq_t = big.tile([P, N_HP], F32)
        for hp in range(N_HP):
            nc.sync.dma_start(
                out=bq_t[:, hp : hp + 1], in_=bq[hp * P : (hp + 1) * P].unsqueeze(-1)
            )
        bv_t = big.tile([P, HF], F32)
        nc.sync.dma_start(out=bv_t[:, :], in_=bv_b[:, :])

        wo_t = big.tile([P, N_HP, C], BF16)
        for hp in range(N_HP):
            wdma[hp % 2].dma_start(out=wo_t[:, hp, :], in_=wo[hp * P : (hp + 1) * P, :])

        v_res = big.tile([P, N_TT, HH * VW], BF16)  # [v_h | 1] per head per chunk
        kt_res = big.tile([P, N_HP, T], BF16)       # K^T (f, t)
        qt = big.tile([P, N_HP, T], BF16)           # Q^T (f, q)
        attout = big.tile([P, N_HP, T], BF16)       # normalized O^T

        v_ones = v_res.rearrange("p t (h w) -> p t h w", w=VW)
        nc.vector.memset(v_ones[:, :, :, D : D + 1], 1.0)

        # ---- V = X @ Wv + bv, all heads (tokens on partitions) --------------
        bv_v = bv_t.rearrange("p (h d) -> p h d", h=HH)
        for tt in range(N_TT):
            if tt % 3 < 2:
                pvf = s_ps.tile([P, 2 * QW], F32, tag="s")
                pv = pvf[:, 0:HF]
            else:
                pv = pr_ps.tile([P, HF], F32, tag="pr")
            for kc in range(N_KC):
                nc.tensor.matmul(
                    pv[:, :],
                    xt[:, kc, tt * P : (tt + 1) * P],
                    wv_t[:, kc, :],
                    start=(kc == 0),
                    stop=(kc == N_KC - 1),
                )
            pv_v = pv.rearrange("p (h d) -> p h d", h=HH)
            nc.vector.tensor_add(v_ones[:, tt, :, 0:D], pv_v[:, :, :], bv_v[:, :, :])

        # ---- projection work generators (emitted inline with attention) ----
        def k_proj_steps(hp, pool=None, tag="pr"):
            """K^T(hp): 4 th-groups x (8 accumulating MMs + a DVE cast)."""
            for th in range(N_QP):
                pk = (pool or pr_ps).tile([P, QW], F32, tag=tag)
                for kc in range(N_KC):
                    yield lambda hp=hp, th=th, kc=kc, pk=pk: nc.tensor.matmul(
                        pk[:, :],
                        wk_t[:, kc, hp * P : (hp + 1) * P],
                        xt[:, kc, th * QW : (th + 1) * QW],
                        start=(kc == 0),
                        stop=(kc == N_KC - 1),
                    )
                yield lambda hp=hp, th=th, pk=pk: nc.vector.tensor_copy(
                    kt_res[:, hp, th * QW : (th + 1) * QW], pk[:, :]
                )

        def q_proj_steps(hp, pool=None, tag="pr"):
            for th in range(N_QP):
                pq = (pool or pr_ps).tile([P, QW], F32, tag=tag)
                for kc in range(N_KC):
                    yield lambda hp=hp, th=th, kc=kc, pq=pq: nc.tensor.matmul(
                        pq[:, :],
                        wq_t[:, kc, hp * P : (hp + 1) * P],
                        xt[:, kc, th * QW : (th + 1) * QW],
                        start=(kc == 0),
                        stop=(kc == N_KC - 1),
                    )
                yield lambda hp=hp, th=th, pq=pq: nc.vector.tensor_scalar_add(
                    qt[:, hp, th * QW : (th + 1) * QW], pq[:, :], bq_t[:, hp : hp + 1]
                )

        odma = [nc.sync, nc.scalar, nc.gpsimd]

        def out_proj_steps(qms, use_s_pool=False):
            """Output projection for query chunks qms (contract all 4 hp)."""
            for qm in qms:
                for nh in range(2):
                    if use_s_pool and (qm + nh) % 2 == 0:
                        # tail only: the s pool is idle after the last ACT
                        po_f = s_ps.tile([P, 2 * QW], F32, tag="s")
                        po = po_f[:, 0:QW]
                    else:
                        po = pr_ps.tile([P, QW], F32, tag="pr")
                    for hp in range(N_HP):
                        yield lambda qm=qm, nh=nh, hp=hp, po=po: nc.tensor.matmul(
                            po[:, :],
                            attout[:, hp, qm * P : (qm + 1) * P],
                            wo_t[:, hp, nh * QW : (nh + 1) * QW],
                            start=(hp == 0),
                            stop=(hp == N_HP - 1),
                        )

                    def _drain(qm=qm, nh=nh, po=po):
                        os_ = bc_pool.tile([P, QW], F32, tag="os")
                        nc.vector.tensor_copy(os_[:, :], po[:, :])
                        odma[(2 * qm + nh) % 3].dma_start(
                            out=out[qm, :, nh * QW : (nh + 1) * QW], in_=os_[:, :]
                        )

                    yield _drain

        def chain(*gens):
            for g in gens:
                yield from g

        # upfront: K^T(0), Q^T(0) (V is already queued above); these use
        # the s pool (idle until attention starts)

        def _s_pool_qw():
            class p:
                @staticmethod
                def tile(shape, dt_, tag=None):
                    t = s_ps.tile([P, 2 * QW], dt_, tag="s")
                    return t[:, 0 : shape[1]]
            return p

        for step in chain(
            k_proj_steps(0, pool=_s_pool_qw()), q_proj_steps(0, pool=_s_pool_qw())
        ):
            step()

        # side work emitted during attention inner loops. The chip power
        # manager allows ~160-200 us of full-rate PE, then duty-cycles the
        # clock to ~0.686 - so ALL projection side work is front-loaded into
        # the first two head pairs (the grace window), leaving hp 2..3 pure
        # attention (~0.69 PE duty, which the clamp tolerates at full pace).
        # Out-proj needs ALL head pairs' attout, so it can only run during
        # hp 3 (pass qp covers chunks of pass qp-1) plus a tail.
        side = {}
        for hp in range(3):
            g = chain(k_proj_steps(hp + 1), q_proj_steps(hp + 1))
            for qp in range(N_QP):
                side[(hp, qp)] = (g, 72 / 56)
        side[(3, 0)] = (iter(()), 0.0)
        for qp in range(1, N_QP):
            side[(3, qp)] = (out_proj_steps(range(4 * (qp - 1), 4 * qp)), 36 / 14)

        # ---- attention: per head pair, per query pass -----------------------
        for hp in range(N_HP):
            hA, hB = 2 * hp, 2 * hp + 1
            for qp in range(N_QP):
                gen, side_per_iter = side[(hp, qp)]
                quota = 0.0
                q0 = qp * QW
                oA = o_ps.tile([VW, QW], F32, tag="o")
                oB = o_ps.tile([VW, QW], F32, tag="o")
                for kt in range(N_TT):
                    s = s_ps.tile([P, 2 * QW], F32, tag="s")
                    nc.tensor.matmul(
                        s[:, 0:QW],
                        kt_res[0:64, hp, kt * P : (kt + 1) * P],
                        qt[0:64, hp, q0 : q0 + QW],
                        start=True,
                        stop=True,
                        tile_position=(0, 0),
                    )
                    nc.tensor.matmul(
                        s[:, QW : 2 * QW],
                        kt_res[64:128, hp, kt * P : (kt + 1) * P],
                        qt[64:128, hp, q0 : q0 + QW],
                        start=True,
                        stop=True,
                        tile_position=(64, 0),
                    )
                    p_t = pt_pool.tile([P, 2 * QW], BF16, tag="pt")
                    nc.scalar.activation(p_t[:, :], s[:, :], AF.Exp, scale=0.125)
                    if debug and hp == 0 and qp == 0 and kt == 0:
                        dcp = bc_pool.tile([P, 2 * QW], F32, tag="dbgs")
                        nc.vector.tensor_copy(dcp[:, :], s[:, :])
                        nc.sync.dma_start(out=dbg["dbg_s"][:, :], in_=dcp[:, :])
                        nc.sync.dma_start(out=dbg["dbg_pt"][:, :], in_=p_t[:, :])
                    # emit interleaved projection work here, between the S
                    # pair and the AV accumulation: the AVs wait on the ACT
                    # anyway, so side MMs slot into that PE gap instead of
                    # queueing behind the AVs and blocking the next S pair
                    if (kt >= 2) if hp == 3 else (kt < 14):
                        quota += side_per_iter
                    while quota >= 1.0:
                        step = next(gen, None)
                        if step is None:
                            quota = 0.0
                            break
                        step()
                        quota -= 1.0
                    nc.tensor.matmul(
                        oA[:, :],
                        v_res[:, kt, hA * VW : (hA + 1) * VW],
                        p_t[:, 0:QW],
                        start=(kt == 0),
                        stop=(kt == N_TT - 1),
                    )
                    nc.tensor.matmul(
                        oB[:, :],
                        v_res[:, kt, hB * VW : (hB + 1) * VW],
                        p_t[:, QW : 2 * QW],
                        start=(kt == 0),
                        stop=(kt == N_TT - 1),
                    )

                if debug and hp == 0 and qp == 0:
                    ocp = bc_pool.tile([VW, QW], F32, tag="dbgo")
                    nc.vector.tensor_copy(ocp[:, :], oA[:, :])
                    nc.sync.dma_start(out=dbg["dbg_o"][:, :], in_=ocp[:, :])
                # normalize: attout[d, q] = O[d, q] / O[64, q].
                # Copy O and den out of PSUM FIRST so the O banks free after
                # ~1.2us (the next pass's AV accumulation reuses them); the
                # recip -> gpsimd-broadcast -> mul chain then runs off the
                # PSUM critical path entirely.
                chains = []
                for row0, o_t in ((0, oA), (64, oB)):
                    o_sb = bc_pool.tile([64, QW], F32, tag="osb")
                    nc.vector.tensor_copy(o_sb[:, :], o_t[0:64, :])
                    den_t = rc_pool.tile([1, QW], F32, tag="den")
                    nc.vector.tensor_copy(den_t[:, :], o_t[64:VW, :])
                    chains.append((row0, o_sb, den_t))
                for row0, o_sb, den_t in chains:
                    rc_t = rc_pool.tile([1, QW], F32, tag="rc")
                    nc.vector.reciprocal_approx_fast(out=rc_t[:, :], in_=den_t[:, :])
                    bc_t = bc_pool.tile([64, QW], F32, tag="bc")
                    nc.gpsimd.partition_broadcast(bc_t[:, :], rc_t[:, :])
                    nc.vector.tensor_mul(
                        attout[row0 : row0 + 64, hp, q0 : q0 + QW],
                        o_sb[:, :],
                        bc_t[:, :],
                    )
                    if debug and hp == 0 and qp == 0 and row0 == 0:
                        nc.sync.dma_start(out=dbg["dbg_rc"][:, :], in_=rc_t[:, :])
                        nc.sync.dma_start(out=dbg["dbg_bc"][:, :], in_=bc_t[:, :])
                # drain leftover side work (shared gens span all 4 passes)
                if qp == N_QP - 1 or hp == 3:
                    for step in gen:
                        step()

        if debug:
            nc.sync.dma_start(out=dbg["dbg_xt"][:, :], in_=xt[:, 0, :])
            nc.sync.dma_start(out=dbg["dbg_qt"][:, :], in_=qt[:, 0, :])
            nc.sync.dma_start(out=dbg["dbg_kt"][:, :], in_=kt_res[:, 0, :])
            nc.sync.dma_start(out=dbg["dbg_vd"][:, :], in_=v_res[:, 0, :])
            nc.sync.dma_start(out=dbg["dbg_ao"][:, :], in_=attout[:, 0, :])

        # ---- output projection tail (qm 12..15; rest ran inside hp 3) -------
        for step in out_proj_steps(range(12, N_TT), use_s_pool=True):
            step()

    nc.finalize()
    return nc


def _get_program():
    if "nc" not in _CACHE:
        _CACHE["nc"] = _build()
    return _CACHE["nc"]


def _bf16(a):
    import ml_dtypes

    return np.asarray(a, np.float32).astype(ml_dtypes.bfloat16)


def kernel(x, Wq, bq, Wk, bk, Wv, bv, Wo, bo, _trace=False, _trace_kwargs=None):
    x = np.asarray(x, np.float32)
    bq, bv, bo = (np.asarray(b, np.float32) for b in (bq, bv, bo))
    # bk unused: a key-side bias adds a per-query constant to every logit of a
    # softmax row, which cancels exactly in the softmax.

    x_b = [np.ascontiguousarray(_bf16(x[b]).T) for b in range(4)]
    wq_h = [_bf16(Wq[:, h * HF : (h + 1) * HF]) for h in range(2)]
    wk_h = [_bf16(Wk[:, h * HF : (h + 1) * HF]) for h in range(2)]
    wv_h = [_bf16(Wv[:, h * HF : (h + 1) * HF]) for h in range(2)]
    wo_h = [np.ascontiguousarray(_bf16(Wo[h * HF : (h + 1) * HF, :])) for h in range(2)]
    bq_h = [np.ascontiguousarray(bq[h * HF : (h + 1) * HF]) for h in range(2)]
    bv_h = [
        np.ascontiguousarray(np.broadcast_to(bv[h * HF : (h + 1) * HF], (P, HF)))
        for h in range(2)
    ]

    nc = _get_program()
    in_maps = []
    for c in range(8):
        b, hh = divmod(c, 2)
        in_maps.append(
            {
                "x": x_b[b],
                "wq": wq_h[hh], "wk": wk_h[hh], "wv": wv_h[hh],
                "wo": wo_h[hh], "bq": bq_h[hh], "bv_b": bv_h[hh],
            }
        )

    kw = {}
    if _trace:
        kw = dict(trace=True, **(_trace_kwargs or {}))
    res = run_bass_kernel_spmd(nc, in_maps, list(range(8)), **kw)
    _CACHE["last_result"] = res

    outp = np.empty((4, T, C), np.float32)
    for b in range(4):
        p0 = res.results[2 * b]["out"].reshape(T, C)
        p1 = res.results[2 * b + 1]["out"].reshape(T, C)
        outp[b] = p0 + p1
    outp += bo.astype(np.float32)
    return outp
